# revision 8
# baseline (speedup 1.0000x reference)
"""Multi-head attention (b=2, s=2048, h=2048, 16 heads x 128) on 8 trn2 cores.

Sharding: core c handles batch c//4 and local head group c%4 (4 heads, 512
hidden cols). Per-core Bass kernel does:
  A) q/k/v projections (weights resident in SBUF, x^T streamed per s-column),
     q^T/k^T stored per head as [d=128, s], v stored natural [s, d_loc];
     spilled to DRAM scratch.
  B) per head: RoPE (DVE), scores^T = k^T.T-style matmul into [j, i] layout so
     the alibi bias is per-partition and folds into the ACT exp; softmax
     denominator comes free from a ones-column appended to v in the attn@v
     matmul; normalize (recip + per-partition mul); PE-transpose to ao^T.
  C) row-parallel output projection -> per-core partial y [s, 2048].
Host: partial sums over each group of 4 cores -> full [2, s, 2048] output.

All matmuls fp32 (1/sqrt(128) folded into Wq host-side).
"""

import os

import numpy as np

import concourse.bass as bass
import concourse.mybir as mybir
import concourse.tile as tile
from concourse.masks import make_identity
from concourse.vector_clock import ScopedClock
from concourse.bass_utils import run_bass_kernel_spmd

DT = mybir.dt.float32
FP = np.float32
S = 2048
HID = 2048
D = 128
NH = 4          # local heads per core
KC = HID // 128  # 16 k-chunks
JC = S // 128    # 16 j-chunks
ISUB = 4         # 128-sub-chunks per 512 i-block
IBLK = 4         # i-blocks of 512

TRACE = bool(int(os.environ.get("KBENCH_TRACE", "0")))
LAST_EXEC_NS = None
LAST_RESULTS = None

_NC = None
MAXW = 1  # this walrus build rejects >1 sem wait per instruction


def _split_excess_waits(nc, maxw=MAXW):
    """Hoist excess sem waits onto same-engine nofuse nops spliced in directly
    before the offending instruction. Pure condition hoisting: the engine
    blocks on each nop's waits before reaching the instruction, so semantics
    are identical."""
    for bb_name, bbw in list(nc.bb_map.items()):
        bb = bbw.bb if hasattr(bbw, "bb") else bbw
        insts = list(bb.instructions)
        changed = False
        out = []
        for inst in insts:
            si = inst.sync_info
            waits = list(si.on_wait) if si is not None and si.on_wait else []
            if len(waits) > maxw and inst.engine in nc.engines:
                si.on_wait = waits[:maxw]
                rest = waits[maxw:]
                for i in range(0, len(rest), maxw):
                    nop = nc.engines[inst.engine].nop(nofuse=True, hint="waitsplit")
                    cb = nc.cur_bb.bb
                    lst = list(cb.instructions)
                    assert lst[-1].name == nop.ins.name
                    cb.instructions = lst[:-1]
                    nop.ins.sync_info = mybir.SyncInfo(
                        on_wait=rest[i : i + maxw], on_update=[]
                    )
                    out.append(nop.ins)
                changed = True
            out.append(inst)
        if changed:
            bb.instructions = out


def build():
    nc = bass.Bass()
    xT = nc.declare_dram_parameter("xT", [HID, S], DT, isOutput=False)
    wqT = nc.declare_dram_parameter("wqT", [HID, NH * D], DT, isOutput=False)
    wkT = nc.declare_dram_parameter("wkT", [HID, NH * D], DT, isOutput=False)
    wvT = nc.declare_dram_parameter("wvT", [HID, NH * D], DT, isOutput=False)
    woT = nc.declare_dram_parameter("woT", [NH * D, HID], DT, isOutput=False)
    cosT = nc.declare_dram_parameter("cosT", [D, S], DT, isOutput=False)
    sinT = nc.declare_dram_parameter("sinT", [D, S], DT, isOutput=False)
    alibi = nc.declare_dram_parameter("alibi", [128, NH * JC], DT, isOutput=False)
    y = nc.declare_dram_parameter("y", [S, HID], DT, isOutput=True)

    qT_d = nc.dram_tensor("qT_d", [NH, D, S], DT)
    kT_d = nc.dram_tensor("kT_d", [NH, D, S], DT)
    v_d = nc.dram_tensor("v_d", [S, NH * D], DT)

    EXP = mybir.ActivationFunctionType.Exp

    with tile.TileContext(nc) as tc:
        with (
            tc.tile_pool(name="ps", bufs=3, space="PSUM") as ps_pool,
            tc.tile_pool(name="pav", bufs=3, space="PSUM") as pav_pool,
            tc.tile_pool(name="pt", bufs=2, space="PSUM") as pt_pool,
            tc.tile_pool(name="persist", bufs=1) as persist,
        ):
            id_t = persist.tile([128, 128], DT, tag="id")
            make_identity(nc, id_t[:])
            ao_t = [
                persist.tile([128, S], DT, tag=f"ao{h}", name=f"ao{h}")
                for h in range(NH)
            ]

            # ---------------- Phase A: q/k/v projections ----------------
            with (
                tc.tile_pool(name="wts", bufs=KC) as wpool,
                tc.tile_pool(name="xp", bufs=KC + 2) as xpool,
                tc.tile_pool(name="stA", bufs=4) as stA,
            ):
                wq_t, wk_t, wv_t = [], [], []
                for kc in range(KC):
                    for lst, src, tg in (
                        (wq_t, wqT, "wq"),
                        (wk_t, wkT, "wk"),
                        (wv_t, wvT, "wv"),
                    ):
                        t = wpool.tile([128, NH * D], DT, tag=tg)
                        nc.sync.dma_start(
                            out=t[:], in_=src[kc * 128 : (kc + 1) * 128, :]
                        )
                        lst.append(t)

                for sc in range(4):  # 512-wide s columns
                    xc = []
                    for kc in range(KC):
                        t = xpool.tile([128, 512], DT, tag="xc")
                        nc.sync.dma_start(
                            out=t[:],
                            in_=xT[kc * 128 : (kc + 1) * 128, sc * 512 : (sc + 1) * 512],
                        )
                        xc.append(t)
                    # q^T, k^T per head: [d=128, s512]
                    for h in range(NH):
                        for wt, dst in ((wq_t, qT_d), (wk_t, kT_d)):
                            ps = ps_pool.tile([128, 512], DT, tag="ps")
                            for kc in range(KC):
                                nc.tensor.matmul(
                                    ps[:],
                                    wt[kc][:, h * D : (h + 1) * D],
                                    xc[kc][:],
                                    start=(kc == 0),
                                    stop=(kc == KC - 1),
                                )
                            st = stA.tile([128, 512], DT, tag="stA")
                            nc.vector.tensor_copy(st[:], ps[:])
                            nc.sync.dma_start(
                                out=dst[h, :, sc * 512 : (sc + 1) * 512], in_=st[:]
                            )
                    # v natural rows: [s128, d_loc 512]
                    for ss in range(4):
                        ps = ps_pool.tile([128, 512], DT, tag="ps")
                        for kc in range(KC):
                            nc.tensor.matmul(
                                ps[:],
                                xc[kc][:, ss * 128 : (ss + 1) * 128],
                                wv_t[kc][:],
                                start=(kc == 0),
                                stop=(kc == KC - 1),
                            )
                        st = stA.tile([128, 512], DT, tag="stA")
                        nc.vector.tensor_copy(st[:], ps[:])
                        nc.sync.dma_start(
                            out=v_d[sc * 512 + ss * 128 : sc * 512 + (ss + 1) * 128, :],
                            in_=st[:],
                        )

            # ---------------- Phase B: attention per head ----------------
            with (
                tc.tile_pool(name="trig", bufs=1) as trig,
                tc.tile_pool(name="qk", bufs=2) as qkpool,
                tc.tile_pool(name="rt", bufs=1) as rtpool,
                tc.tile_pool(name="vv", bufs=JC + 2) as vpool,
                tc.tile_pool(name="wexp", bufs=JC + 1) as wpool2,
                tc.tile_pool(name="sm", bufs=4) as smallpool,
            ):
                cos_t = trig.tile([D, S], DT, tag="cos")
                nc.sync.dma_start(out=cos_t[:], in_=cosT[:])
                # sinT param carries host-precomputed +/- sin (rows 0..63
                # negated), so rope is: dst = q*cos + qswap*sinpm with qswap
                # loaded partition-rotated straight from DRAM.
                sin_t = trig.tile([D, S], DT, tag="sin")
                nc.sync.dma_start(out=sin_t[:], in_=sinT[:])
                al_t = trig.tile([128, NH * JC], DT, tag="al")
                nc.sync.dma_start(out=al_t[:], in_=alibi[:])

                def rope(src_d, h, ld_tag, dst_tag):
                    ld = qkpool.tile([D, S], DT, tag=ld_tag, name=ld_tag, bufs=1)
                    nc.sync.dma_start(out=ld[:], in_=src_d[h, :, :])
                    sw = qkpool.tile([D, S], DT, tag=ld_tag + "sw", name=ld_tag + "sw", bufs=1)
                    nc.sync.dma_start(out=sw[0:64, :], in_=src_d[h, 64:128, :])
                    nc.sync.dma_start(out=sw[64:128, :], in_=src_d[h, 0:64, :])
                    tmp = rtpool.tile([D, S], DT, tag="rtmp")
                    nc.vector.tensor_mul(tmp[:], sw[:], sin_t[:])
                    dst = qkpool.tile([D, S], DT, tag=dst_tag, name=dst_tag)
                    nc.vector.tensor_mul(dst[:], ld[:], cos_t[:])
                    nc.vector.tensor_add(dst[:], dst[:], tmp[:])
                    return dst

                for h in range(NH):
                    qr = rope(qT_d, h, "qld", "qr")
                    kr = rope(kT_d, h, "kld", "kr")
                    vts = []
                    for jc in range(JC):
                        vt = vpool.tile([128, 132], DT, tag="vt")
                        nc.sync.dma_start(
                            out=vt[:, 0:128],
                            in_=v_d[jc * 128 : (jc + 1) * 128, h * D : (h + 1) * D],
                        )
                        nc.vector.memset(vt[:, 128:129], 1.0)
                        vts.append(vt)
                    for ib in range(IBLK):
                        wts_l = []
                        for jc in range(JC):
                            ps = ps_pool.tile([128, 512], DT, tag="ps")
                            nc.tensor.matmul(
                                ps[:],
                                kr[:, jc * 128 : (jc + 1) * 128],
                                qr[:, ib * 512 : (ib + 1) * 512],
                                start=True,
                                stop=True,
                            )
                            w = wpool2.tile([128, 512], DT, tag="w")
                            nc.scalar.activation(
                                w[:],
                                ps[:],
                                EXP,
                                bias=al_t[:, h * JC + jc : h * JC + jc + 1],
                                scale=1.0,
                            )
                            wts_l.append(w)
                        for isub in range(ISUB):
                            pav = pav_pool.tile([128, 132], DT, tag="pav")
                            for jc in range(JC):
                                nc.tensor.matmul(
                                    pav[:, 0:129],
                                    wts_l[jc][:, isub * 128 : (isub + 1) * 128],
                                    vts[jc][:, 0:129],
                                    start=(jc == 0),
                                    stop=(jc == JC - 1),
                                )
                            rec = smallpool.tile([128, 1], DT, tag="rec")
                            nc.vector.reciprocal(rec[:], pav[:, 128:129])
                            onrm = smallpool.tile([128, 128], DT, tag="onrm")
                            nc.vector.tensor_scalar_mul(onrm[:], pav[:, 0:128], rec[:])
                            pt = pt_pool.tile([128, 128], DT, tag="pt")
                            nc.tensor.transpose(pt[:], onrm[:], id_t[:])
                            c0 = ib * 512 + isub * 128
                            nc.vector.tensor_copy(ao_t[h][:, c0 : c0 + 128], pt[:])

            # ---------------- Phase C: output projection ----------------
            with (
                tc.tile_pool(name="wo", bufs=1) as wopool,
                tc.tile_pool(name="stC", bufs=4) as stC,
            ):
                wo_t = []
                for cc in range(NH):
                    t = wopool.tile([128, HID], DT, tag=f"wo{cc}")
                    nc.sync.dma_start(out=t[:], in_=woT[cc * 128 : (cc + 1) * 128, :])
                    wo_t.append(t)
                for scn in range(S // 128):
                    for ocn in range(4):
                        ps = ps_pool.tile([128, 512], DT, tag="ps")
                        for cc in range(NH):
                            nc.tensor.matmul(
                                ps[:],
                                ao_t[cc][:, scn * 128 : (scn + 1) * 128],
                                wo_t[cc][:, ocn * 512 : (ocn + 1) * 512],
                                start=(cc == 0),
                                stop=(cc == NH - 1),
                            )
                        st = stC.tile([128, 512], DT, tag="stC")
                        nc.vector.tensor_copy(st[:], ps[:])
                        nc.sync.dma_start(
                            out=y[scn * 128 : (scn + 1) * 128, ocn * 512 : (ocn + 1) * 512],
                            in_=st[:],
                        )
    _split_excess_waits(nc)
    return nc


def _get_nc():
    global _NC
    if _NC is None:
        _NC = build()
    return _NC


def _numpy_fallback(x, attention_mask, alibi, freqs, Wq, Wk, Wv, Wo):
    b, s, hidden = x.shape
    H, d = 16, 128

    def proj(W):
        yv = x @ W.T
        return yv.reshape(b, s, H, d).transpose(0, 2, 1, 3)

    q, k, v = proj(Wq), proj(Wk), proj(Wv)
    cos, sin = np.cos(freqs), np.sin(freqs)

    def rot(t):
        t1, t2 = t[..., :64], t[..., 64:]
        return np.concatenate((-t2, t1), axis=-1)

    q = q * cos + rot(q) * sin
    k = k * cos + rot(k) * sin
    scores = np.einsum("bhqd,bhkd->bhqk", q, k) / np.sqrt(d)
    scores = scores + attention_mask + alibi
    m = scores.max(axis=-1, keepdims=True)
    e = np.exp(scores - m)
    attn = e / e.sum(axis=-1, keepdims=True)
    out = np.einsum("bhqk,bhkd->bhqd", attn, v)
    out = out.transpose(0, 2, 1, 3).reshape(b, s, hidden)
    return (out @ Wo.T).astype(np.float32)


def kernel(x, attention_mask, alibi, freqs, Wq, Wk, Wv, Wo):
    global LAST_EXEC_NS, LAST_RESULTS
    x = np.asarray(x, dtype=FP)
    attention_mask = np.asarray(attention_mask, dtype=FP)
    alibi = np.asarray(alibi, dtype=FP)
    freqs = np.asarray(freqs, dtype=FP)
    Wq, Wk, Wv, Wo = (np.asarray(w, dtype=FP) for w in (Wq, Wk, Wv, Wo))

    if np.any(attention_mask):
        return _numpy_fallback(x, attention_mask, alibi, freqs, Wq, Wk, Wv, Wo)

    nc = _get_nc()

    f = freqs.reshape(S, D)
    cosT = np.ascontiguousarray(np.cos(f).T)
    sinT = np.ascontiguousarray(np.sin(f).T)
    sinT[0:64, :] *= -1.0  # rotate_half sign folded into sin
    xTs = [np.ascontiguousarray(x[b].T) for b in range(2)]
    scale = FP(1.0 / np.sqrt(D))

    in_maps = []
    for c in range(8):
        b, g = divmod(c, 4)
        r0, r1 = g * 512, (g + 1) * 512
        wqT = np.ascontiguousarray(Wq[r0:r1, :].T) * scale
        wkT = np.ascontiguousarray(Wk[r0:r1, :].T)
        wvT = np.ascontiguousarray(Wv[r0:r1, :].T)
        woT = np.ascontiguousarray(Wo[:, r0:r1].T)
        al_loc = alibi[0, g * NH : (g + 1) * NH, 0, :]  # [4, 2048]
        al_dev = np.ascontiguousarray(
            al_loc.reshape(NH, JC, 128).transpose(2, 0, 1).reshape(128, NH * JC)
        )
        in_maps.append(
            {
                "xT": xTs[b],
                "wqT": wqT,
                "wkT": wkT,
                "wvT": wvT,
                "woT": woT,
                "cosT": cosT,
                "sinT": sinT,
                "alibi": al_dev,
            }
        )

    res = run_bass_kernel_spmd(nc, in_maps, list(range(8)), trace=TRACE)
    LAST_EXEC_NS = res.exec_time_ns
    LAST_RESULTS = res
    ys = [res.results[c]["y"] for c in range(8)]
    out = np.stack(
        [ys[0] + ys[1] + ys[2] + ys[3], ys[4] + ys[5] + ys[6] + ys[7]], axis=0
    )
    return out.astype(np.float32)


# revision 22
# speedup vs baseline: 2.3437x; 2.3437x over previous
"""Multi-head attention (b=2, s=2048, h=2048, 16 heads x 128) on 8 trn2 cores.

Sharding: core c handles batch c//4 and local head group c%4 (4 heads, 512
hidden cols). Per-core Bass kernel does:
  A) q/k/v projections (weights resident in SBUF, x^T streamed per s-column),
     q^T/k^T stored per head as [d=128, s], v stored natural [s, d_loc];
     spilled to DRAM scratch.
  B) per head: RoPE (DVE), scores^T = k^T.T-style matmul into [j, i] layout so
     the alibi bias is per-partition and folds into the ACT exp; softmax
     denominator comes free from a ones-column appended to v in the attn@v
     matmul; normalize (recip + per-partition mul); PE-transpose to ao^T.
  C) row-parallel output projection -> per-core partial y [s, 2048].
Host: partial sums over each group of 4 cores -> full [2, s, 2048] output.

All matmuls fp32 (1/sqrt(128) folded into Wq host-side).
"""

import os

import numpy as np

import concourse.bass as bass
import concourse.mybir as mybir
import concourse.tile as tile
from concourse.masks import make_identity
from concourse.vector_clock import ScopedClock
from concourse.bass_utils import run_bass_kernel_spmd

DT = mybir.dt.float32
FP = np.float32
S = 2048
HID = 2048
D = 128
NH = 4          # local heads per core
KC = HID // 128  # 16 k-chunks
JC = S // 128    # 16 j-chunks
ISUB = 4         # 128-sub-chunks per 512 i-block
IBLK = 4         # i-blocks of 512

TRACE = bool(int(os.environ.get("KBENCH_TRACE", "0")))
F32R = bool(int(os.environ.get("KBENCH_F32R", "0")))
AVN = 260 if F32R else 129  # attn@v matmul free dim (f32r needs >=256)
DTM = mybir.dt.float32r if F32R else mybir.dt.float32  # matmul-input dtype
LAST_EXEC_NS = None
LAST_RESULTS = None

_NC = None
MAXW = 1  # this walrus build rejects >1 sem wait per instruction


def _split_excess_waits(nc, maxw=MAXW):
    """Hoist excess sem waits onto same-engine nofuse nops spliced in directly
    before the offending instruction. Pure condition hoisting: the engine
    blocks on each nop's waits before reaching the instruction, so semantics
    are identical."""
    for bb_name, bbw in list(nc.bb_map.items()):
        bb = bbw.bb if hasattr(bbw, "bb") else bbw
        insts = list(bb.instructions)
        changed = False
        out = []
        for inst in insts:
            si = inst.sync_info
            waits = list(si.on_wait) if si is not None and si.on_wait else []
            if len(waits) > maxw and inst.engine in nc.engines:
                si.on_wait = waits[:maxw]
                rest = waits[maxw:]
                for i in range(0, len(rest), maxw):
                    nop = nc.engines[inst.engine].nop(nofuse=True, hint="waitsplit")
                    cb = nc.cur_bb.bb
                    lst = list(cb.instructions)
                    assert lst[-1].name == nop.ins.name
                    cb.instructions = lst[:-1]
                    nop.ins.sync_info = mybir.SyncInfo(
                        on_wait=rest[i : i + maxw], on_update=[]
                    )
                    out.append(nop.ins)
                changed = True
            out.append(inst)
        if changed:
            bb.instructions = out


def build():
    nc = bass.Bass()
    xT = nc.declare_dram_parameter("xT", [HID, S], DTM, isOutput=False)
    wqT = nc.declare_dram_parameter("wqT", [HID, NH * D], DTM, isOutput=False)
    wkT = nc.declare_dram_parameter("wkT", [HID, NH * D], DTM, isOutput=False)
    wvT = nc.declare_dram_parameter("wvT", [HID, NH * D], DTM, isOutput=False)
    woT = nc.declare_dram_parameter("woT", [NH * D, HID], DTM, isOutput=False)
    cosT = nc.declare_dram_parameter("cosT", [D, S], DT, isOutput=False)
    sinT = nc.declare_dram_parameter("sinT", [D, S], DT, isOutput=False)
    alibi = nc.declare_dram_parameter("alibi", [128, NH * JC], DT, isOutput=False)
    # ones column (softmax denominator) + zero padding for the attn@v rhs
    vpad = nc.declare_dram_parameter("vpad", [128, AVN - 128], DTM, isOutput=False)
    y = nc.declare_dram_parameter("y", [S, HID], DT, isOutput=True)

    qT_d = nc.dram_tensor("qT_d", [NH, D, S], DTM)
    kT_d = nc.dram_tensor("kT_d", [NH, D, S], DTM)
    v_d = nc.dram_tensor("v_d", [S, NH * D], DTM)

    EXP = mybir.ActivationFunctionType.Exp

    def mm(out, lhsT, rhs, start, stop):
        nc.tensor.matmul(out, lhsT, rhs, start=start, stop=stop)

    with tile.TileContext(nc) as tc:
        with (
            tc.tile_pool(name="ps", bufs=3, space="PSUM") as ps_pool,
            tc.tile_pool(name="pav", bufs=3, space="PSUM") as pav_pool,
            tc.tile_pool(name="pt", bufs=2, space="PSUM") as pt_pool,
            tc.tile_pool(name="persist", bufs=1) as persist,
        ):
            id_t = persist.tile([128, 128], DT, tag="id")
            make_identity(nc, id_t[:])
            ao_t = [
                persist.tile([128, S], DTM, tag=f"ao{h}", name=f"ao{h}")
                for h in range(NH)
            ]

            # ---------------- Phase A: q/k/v projections ----------------
            with (
                tc.tile_pool(name="wts", bufs=KC) as wpool,
                tc.tile_pool(name="xp", bufs=KC + 2) as xpool,
                tc.tile_pool(name="stA", bufs=4) as stA,
            ):
                wq_t, wk_t, wv_t = [], [], []
                for kc in range(KC):
                    for lst, src, tg in (
                        (wq_t, wqT, "wq"),
                        (wk_t, wkT, "wk"),
                        (wv_t, wvT, "wv"),
                    ):
                        t = wpool.tile([128, NH * D], DTM, tag=tg)
                        nc.sync.dma_start(
                            out=t[:], in_=src[kc * 128 : (kc + 1) * 128, :]
                        )
                        lst.append(t)

                for sc in range(4):  # 512-wide s columns
                    xc = []
                    for kc in range(KC):
                        t = xpool.tile([128, 512], DTM, tag="xc")
                        nc.sync.dma_start(
                            out=t[:],
                            in_=xT[kc * 128 : (kc + 1) * 128, sc * 512 : (sc + 1) * 512],
                        )
                        xc.append(t)
                    # q^T, k^T per head: [d=128, s512]
                    for h in range(NH):
                        for wt, dst in ((wq_t, qT_d), (wk_t, kT_d)):
                            ps = ps_pool.tile([128, 512], DT, tag="ps")
                            for kc in range(KC):
                                mm(
                                    ps[:],
                                    wt[kc][:, h * D : (h + 1) * D],
                                    xc[kc][:],
                                    start=(kc == 0),
                                    stop=(kc == KC - 1),
                                )
                            st = stA.tile([128, 512], DTM, tag="stA")
                            nc.vector.tensor_copy(st[:], ps[:])
                            nc.sync.dma_start(
                                out=dst[h, :, sc * 512 : (sc + 1) * 512], in_=st[:]
                            )
                    # v natural rows: [s128, d_loc 512]
                    for ss in range(4):
                        ps = ps_pool.tile([128, 512], DT, tag="ps")
                        for kc in range(KC):
                            mm(
                                ps[:],
                                xc[kc][:, ss * 128 : (ss + 1) * 128],
                                wv_t[kc][:],
                                start=(kc == 0),
                                stop=(kc == KC - 1),
                            )
                        st = stA.tile([128, 512], DTM, tag="stA")
                        nc.vector.tensor_copy(st[:], ps[:])
                        nc.sync.dma_start(
                            out=v_d[sc * 512 + ss * 128 : sc * 512 + (ss + 1) * 128, :],
                            in_=st[:],
                        )

            # ---------------- Phase B: attention per head ----------------
            with (
                tc.tile_pool(name="trig", bufs=1) as trig,
                tc.tile_pool(name="qk", bufs=2) as qkpool,
                tc.tile_pool(name="rt", bufs=1) as rtpool,
                tc.tile_pool(name="vv", bufs=JC + 2) as vpool,
                tc.tile_pool(name="wexp", bufs=JC + 1) as wpool2,
                tc.tile_pool(name="sm", bufs=4) as smallpool,
            ):
                cos_t = trig.tile([D, S], DT, tag="cos")
                nc.sync.dma_start(out=cos_t[:], in_=cosT[:])
                # sinT param carries host-precomputed +/- sin (rows 0..63
                # negated), so rope is: dst = q*cos + qswap*sinpm with qswap
                # loaded partition-rotated straight from DRAM.
                sin_t = trig.tile([D, S], DT, tag="sin")
                nc.sync.dma_start(out=sin_t[:], in_=sinT[:])
                al_t = trig.tile([128, NH * JC], DT, tag="al")
                nc.sync.dma_start(out=al_t[:], in_=alibi[:])

                def rope(src_d, h, ld_tag, dst_tag):
                    ld = qkpool.tile([D, S], DT, tag=ld_tag, name=ld_tag, bufs=1)
                    nc.sync.dma_start(out=ld[:], in_=src_d[h, :, :].bitcast(DT))
                    sw = qkpool.tile([D, S], DT, tag=ld_tag + "sw", name=ld_tag + "sw", bufs=1)
                    nc.sync.dma_start(out=sw[0:64, :], in_=src_d[h, 64:128, :].bitcast(DT))
                    nc.sync.dma_start(out=sw[64:128, :], in_=src_d[h, 0:64, :].bitcast(DT))
                    tmp = rtpool.tile([D, S], DT, tag="rtmp")
                    nc.vector.tensor_mul(tmp[:], sw[:], sin_t[:])
                    dst = qkpool.tile([D, S], DTM, tag=dst_tag, name=dst_tag)
                    nc.vector.tensor_mul(dst[:], ld[:], cos_t[:])
                    nc.vector.tensor_add(dst[:], dst[:], tmp[:])
                    return dst

                for h in range(NH):
                    qr = rope(qT_d, h, "qld", "qr")
                    kr = rope(kT_d, h, "kld", "kr")
                    # v_aug: cols 0:128 = v, col 128 = ones (softmax denom),
                    # cols 129:AVN = zeros (pad so f32r matmul N >= 256).
                    vts = []
                    for jc in range(JC):
                        vt = vpool.tile([128, AVN], DTM, tag="vt")
                        nc.sync.dma_start(
                            out=vt[:, 0:128],
                            in_=v_d[jc * 128 : (jc + 1) * 128, h * D : (h + 1) * D],
                        )
                        nc.sync.dma_start(out=vt[:, 128:AVN], in_=vpad[:])
                        vts.append(vt)
                    for ib in range(IBLK):
                        wts_l = []
                        for jc in range(JC):
                            ps = ps_pool.tile([128, 512], DT, tag="ps")
                            mm(
                                ps[:],
                                kr[:, jc * 128 : (jc + 1) * 128],
                                qr[:, ib * 512 : (ib + 1) * 512],
                                start=True,
                                stop=True,
                            )
                            w = wpool2.tile([128, 512], DTM, tag="w")
                            nc.scalar.activation(
                                w[:],
                                ps[:],
                                EXP,
                                bias=al_t[:, h * JC + jc : h * JC + jc + 1],
                                scale=1.0,
                            )
                            wts_l.append(w)
                        for isub in range(ISUB):
                            pav = pav_pool.tile([128, AVN], DT, tag="pav")
                            for jc in range(JC):
                                mm(
                                    pav[:, 0:AVN],
                                    wts_l[jc][:, isub * 128 : (isub + 1) * 128],
                                    vts[jc][:, 0:AVN],
                                    start=(jc == 0),
                                    stop=(jc == JC - 1),
                                )
                            rec = smallpool.tile([128, 1], DT, tag="rec")
                            nc.vector.reciprocal(rec[:], pav[:, 128:129])
                            onrm = smallpool.tile([128, 128], DT, tag="onrm")
                            nc.vector.tensor_scalar_mul(onrm[:], pav[:, 0:128], rec[:])
                            pt = pt_pool.tile([128, 128], DT, tag="pt")
                            nc.tensor.transpose(pt[:], onrm[:], id_t[:])
                            c0 = ib * 512 + isub * 128
                            nc.vector.tensor_copy(ao_t[h][:, c0 : c0 + 128], pt[:])

            # ---------------- Phase C: output projection ----------------
            with (
                tc.tile_pool(name="wo", bufs=1) as wopool,
                tc.tile_pool(name="stC", bufs=4) as stC,
            ):
                wo_t = []
                for cc in range(NH):
                    t = wopool.tile([128, HID], DTM, tag=f"wo{cc}")
                    nc.sync.dma_start(out=t[:], in_=woT[cc * 128 : (cc + 1) * 128, :])
                    wo_t.append(t)
                for scn in range(S // 128):
                    for ocn in range(4):
                        ps = ps_pool.tile([128, 512], DT, tag="ps")
                        for cc in range(NH):
                            mm(
                                ps[:],
                                ao_t[cc][:, scn * 128 : (scn + 1) * 128],
                                wo_t[cc][:, ocn * 512 : (ocn + 1) * 512],
                                start=(cc == 0),
                                stop=(cc == NH - 1),
                            )
                        st = stC.tile([128, 512], DT, tag="stC")
                        nc.vector.tensor_copy(st[:], ps[:])
                        nc.sync.dma_start(
                            out=y[scn * 128 : (scn + 1) * 128, ocn * 512 : (ocn + 1) * 512],
                            in_=st[:],
                        )
    _split_excess_waits(nc)
    return nc


def _get_nc():
    global _NC
    if _NC is None:
        _NC = build()
    return _NC


def _numpy_fallback(x, attention_mask, alibi, freqs, Wq, Wk, Wv, Wo):
    b, s, hidden = x.shape
    H, d = 16, 128

    def proj(W):
        yv = x @ W.T
        return yv.reshape(b, s, H, d).transpose(0, 2, 1, 3)

    q, k, v = proj(Wq), proj(Wk), proj(Wv)
    cos, sin = np.cos(freqs), np.sin(freqs)

    def rot(t):
        t1, t2 = t[..., :64], t[..., 64:]
        return np.concatenate((-t2, t1), axis=-1)

    q = q * cos + rot(q) * sin
    k = k * cos + rot(k) * sin
    scores = np.einsum("bhqd,bhkd->bhqk", q, k) / np.sqrt(d)
    scores = scores + attention_mask + alibi
    m = scores.max(axis=-1, keepdims=True)
    e = np.exp(scores - m)
    attn = e / e.sum(axis=-1, keepdims=True)
    out = np.einsum("bhqk,bhkd->bhqd", attn, v)
    out = out.transpose(0, 2, 1, 3).reshape(b, s, hidden)
    return (out @ Wo.T).astype(np.float32)


def kernel(x, attention_mask, alibi, freqs, Wq, Wk, Wv, Wo):
    global LAST_EXEC_NS, LAST_RESULTS
    x = np.asarray(x, dtype=FP)
    attention_mask = np.asarray(attention_mask, dtype=FP)
    alibi = np.asarray(alibi, dtype=FP)
    freqs = np.asarray(freqs, dtype=FP)
    Wq, Wk, Wv, Wo = (np.asarray(w, dtype=FP) for w in (Wq, Wk, Wv, Wo))

    if np.any(attention_mask):
        return _numpy_fallback(x, attention_mask, alibi, freqs, Wq, Wk, Wv, Wo)

    nc = _get_nc()

    f = freqs.reshape(S, D)
    cosT = np.ascontiguousarray(np.cos(f).T)
    sinT = np.ascontiguousarray(np.sin(f).T)
    sinT[0:64, :] *= -1.0  # rotate_half sign folded into sin
    vpad = np.zeros((128, AVN - 128), dtype=FP)
    vpad[:, 0] = 1.0
    xTs = [np.ascontiguousarray(x[b].T) for b in range(2)]
    scale = FP(1.0 / np.sqrt(D))

    in_maps = []
    for c in range(8):
        b, g = divmod(c, 4)
        r0, r1 = g * 512, (g + 1) * 512
        wqT = np.ascontiguousarray(Wq[r0:r1, :].T) * scale
        wkT = np.ascontiguousarray(Wk[r0:r1, :].T)
        wvT = np.ascontiguousarray(Wv[r0:r1, :].T)
        woT = np.ascontiguousarray(Wo[:, r0:r1].T)
        al_loc = alibi[0, g * NH : (g + 1) * NH, 0, :]  # [4, 2048]
        al_dev = np.ascontiguousarray(
            al_loc.reshape(NH, JC, 128).transpose(2, 0, 1).reshape(128, NH * JC)
        )
        in_maps.append(
            {
                "xT": xTs[b],
                "wqT": wqT,
                "wkT": wkT,
                "wvT": wvT,
                "woT": woT,
                "cosT": cosT,
                "sinT": sinT,
                "alibi": al_dev,
                "vpad": vpad,
            }
        )

    res = run_bass_kernel_spmd(nc, in_maps, list(range(8)), trace=TRACE)
    LAST_EXEC_NS = res.exec_time_ns
    LAST_RESULTS = res
    ys = [res.results[c]["y"] for c in range(8)]
    out = np.stack(
        [ys[0] + ys[1] + ys[2] + ys[3], ys[4] + ys[5] + ys[6] + ys[7]], axis=0
    )
    return out.astype(np.float32)
